# revision 6
# baseline (speedup 1.0000x reference)
"""ActorCriticLoss (TD-lambda + symlog critic) on 8 Trainium2 NeuronCores.

Data-parallel: the batch axis (65536) is sharded 8 ways. Each core runs an
identical Bass/Tile kernel over its (8192, 64) shard:

  phi_t = (r_t + (K1/K2) * v_t) + K2 * c_t * phi_{t+1}        (backward scan)
  ret_t = phi_t - (K1/K2) * v_t

(the substitution phi = ret + (K1/K2)*v cancels the c*v_next product of the
naive recurrence, so no shifted-values product pass is needed). The backward
recurrence runs as a single forward `tensor_tensor_scan` per tile over a
per-row padded, time-reversed stream: each row contributes 65 slots
[pad, t=63, ..., t=0] with k_pad=0 and a_pad=bootstrap*(1+K1/K2), so the
fp32 scan state resets to the bootstrap value at every row boundary and one
instruction scans all rows in a partition.

Each core reduces its shard to per-partition partial sums / extrema
(sum lp*ret, sum lp, sum lp*v, sum entropy, sum (symlog v - symlog ret)^2,
min ret, max ret); the final normalization (global min/max EMA) and loss
assembly are O(1) and run on the host in float64.
"""

import sys

import numpy as np

sys.path.insert(0, "/opt/trn_rl_repo")

import concourse.bass as bass  # noqa: E402
import concourse.mybir as mybir  # noqa: E402
import concourse.tile as tile  # noqa: E402
from concourse import bacc  # noqa: E402
from concourse.bass_utils import run_bass_kernel_spmd  # noqa: E402

# Problem shape (hardcoded; kernel.py must be self-contained).
B, T = 65536, 64
NCORES = 8
B_LOC = B // NCORES          # 8192 rows per core
P = 128                      # SBUF partitions
M = 16                       # batch rows per partition per tile
NT = B_LOC // (P * M)        # 4 tiles per core
F = M * T                    # 1024 elements/partition per tile
S = T + 1                    # padded slots per row (pad + 64 time steps)
FP = M * S                   # 1040 padded elements/partition

DISCOUNT, LAMBDA = 0.997, 0.95
ENTROPY_SCALE = 0.0003
RETURN_EMA_DECAY = 0.99
K2 = DISCOUNT * LAMBDA                 # coefficient on phi_{t+1}
RATIO = (1.0 - LAMBDA) / LAMBDA        # K1/K2
SIGN_MASK = 0x80000000

f32 = mybir.dt.float32
u32 = mybir.dt.uint32
AX = mybir.AxisListType
OP = mybir.AluOpType
AF = mybir.ActivationFunctionType

# acc_dve columns: u1(0:NT) u2(NT:2NT) mx(2NT:3NT) mn(3NT:4NT)
# acc_act columns: slp(0:NT) sent(NT:2NT) d2(2NT:3NT)
N_DVE = 4 * NT
N_ACT = 3 * NT
N_OUT = N_DVE + N_ACT


def _stt_uint_imm(eng, out, in0, imm, in1, op0, op1):
    """scalar_tensor_tensor with a uint32-typed immediate.

    The public wrapper always emits float32 immediates; the BIR verifier
    requires bitvec-op immediates to be integer-typed and match src/dst.
    """
    return eng.add_instruction(
        mybir.InstTensorScalarPtr(
            name=eng.bass.get_next_instruction_name(),
            is_scalar_tensor_tensor=True,
            op0=op0,
            op1=op1,
            ins=[
                eng.lower_ap(in0),
                mybir.ImmediateValue(dtype=u32, value=imm),
                eng.lower_ap(in1),
            ],
            outs=[eng.lower_ap(out)],
        )
    )


def build_module():
    nc = bacc.Bacc(
        "TRN2", target_bir_lowering=False, debug=False, enable_asserts=False
    )
    r_d = nc.dram_tensor("rewards", [B_LOC, T], f32, kind="ExternalInput").ap()
    v_d = nc.dram_tensor("values", [B_LOC, T], f32, kind="ExternalInput").ap()
    c_d = nc.dram_tensor("continues", [B_LOC, T], f32, kind="ExternalInput").ap()
    bs_d = nc.dram_tensor("bootstrap", [B_LOC], f32, kind="ExternalInput").ap()
    lp_d = nc.dram_tensor("log_probs", [B_LOC, T], f32, kind="ExternalInput").ap()
    en_d = nc.dram_tensor("entropy", [B_LOC, T], f32, kind="ExternalInput").ap()
    out_d = nc.dram_tensor("out", [P, N_OUT], f32, kind="ExternalOutput").ap()

    # DRAM views: row b = n*(P*M) + p*M + m; per-partition lines are M
    # contiguous rows of T = 4KB.
    r_v = r_d.rearrange("(n p m) t -> n p (m t)", p=P, m=M)
    v_v = v_d.rearrange("(n p m) t -> n p (m t)", p=P, m=M)
    c_v = c_d.rearrange("(n p m) t -> n p (m t)", p=P, m=M)
    lp_v = lp_d.rearrange("(n p m) t -> n p (m t)", p=P, m=M)
    en_v = en_d.rearrange("(n p m) t -> n p (m t)", p=P, m=M)
    bs_v = bs_d.rearrange("(n p m) -> n p m", p=P, m=M)

    with tile.TileContext(nc) as tc:
        with (
            tc.tile_pool(name="ins", bufs=2) as ins,
            tc.tile_pool(name="work", bufs=2) as work,
            tc.tile_pool(name="accp", bufs=1) as accp,
        ):
            acc_dve = accp.tile([P, N_DVE], f32)
            acc_act = accp.tile([P, N_ACT], f32)

            for n in range(NT):
                r_t = ins.tile([P, F], f32)
                v_t = ins.tile([P, F], f32)
                c_t = ins.tile([P, F], f32)
                lp_t = ins.tile([P, F], f32)
                en_t = ins.tile([P, F], f32)
                bs_t = ins.tile([P, M], f32)
                nc.sync.dma_start(r_t[:], r_v[n])
                nc.sync.dma_start(v_t[:], v_v[n])
                nc.sync.dma_start(c_t[:], c_v[n])
                nc.sync.dma_start(lp_t[:], lp_v[n])
                nc.sync.dma_start(en_t[:], en_v[n])
                nc.sync.dma_start(bs_t[:], bs_v[n])

                vs_t = work.tile([P, F], f32)
                a_t = work.tile([P, FP], f32)
                k_t = work.tile([P, FP], f32)
                phi_t = work.tile([P, FP], f32)
                ret_t = work.tile([P, F], f32)
                av_t = work.tile([P, F], f32)
                lnv_t = work.tile([P, F], f32)
                ar_t = work.tile([P, F], f32)
                lnr_t = work.tile([P, F], f32)
                d_t = work.tile([P, F], f32)
                j1_t = work.tile([P, F], f32)
                ja_t = work.tile([P, F], f32)

                r3 = r_t[:].rearrange("p (m t) -> p m t", t=T)
                vs3 = vs_t[:].rearrange("p (m t) -> p m t", t=T)
                c3 = c_t[:].rearrange("p (m t) -> p m t", t=T)
                a3 = a_t[:].rearrange("p (m s) -> p m s", s=S)
                k3 = k_t[:].rearrange("p (m s) -> p m s", s=S)
                phi3 = phi_t[:].rearrange("p (m s) -> p m s", s=S)
                ret3 = ret_t[:].rearrange("p (m t) -> p m t", t=T)
                a_rev = a3[:, :, 1:S][:, :, ::-1]
                k_rev = k3[:, :, 1:S][:, :, ::-1]
                phi_nat = phi3[:, :, T:0:-1]

                # ACT: vs = v * RATIO
                nc.scalar.activation(vs_t[:], v_t[:], AF.Copy, scale=RATIO)
                # Pool: a = vs + r (reversed into padded stream);
                # DVE: pad slot = bootstrap * (1 + RATIO)
                nc.gpsimd.tensor_add(a_rev, vs3, r3)
                nc.vector.tensor_scalar_mul(
                    a3[:, :, 0:1], bs_t[:].unsqueeze(2), 1.0 + RATIO
                )
                # DVE: k = K2*c (reversed); Pool: pad slot = 0
                nc.vector.tensor_scalar_mul(k_rev, c3, K2)
                nc.gpsimd.memset(k3[:, :, 0:1], 0.0)
                # DVE: one backward TD(lambda) scan for all rows in the tile
                nc.vector.tensor_tensor_scan(
                    phi_t[:], k_t[:], a_t[:], 0.0, OP.mult, OP.add
                )
                # Pool: ret = phi - vs (natural time order)
                nc.gpsimd.tensor_sub(ret3, phi_nat, vs3)
                # DVE: max/min of returns
                nc.vector.tensor_reduce(
                    acc_dve[:, 2 * NT + n : 2 * NT + n + 1],
                    ret_t[:],
                    axis=AX.X,
                    op=OP.max,
                )
                nc.vector.tensor_reduce(
                    acc_dve[:, 3 * NT + n : 3 * NT + n + 1],
                    ret_t[:],
                    axis=AX.X,
                    op=OP.min,
                )
                # DVE: sum(lp*ret), sum(lp*v)
                nc.vector.scalar_tensor_tensor(
                    j1_t[:], lp_t[:], 1.0, ret_t[:], OP.mult, OP.mult,
                    accum_out=acc_dve[:, n : n + 1],
                )
                nc.vector.scalar_tensor_tensor(
                    j1_t[:], lp_t[:], 1.0, v_t[:], OP.mult, OP.mult,
                    accum_out=acc_dve[:, NT + n : NT + n + 1],
                )
                # ACT: symlog magnitudes + input sums
                nc.scalar.activation(av_t[:], v_t[:], AF.Abs)
                nc.scalar.activation(lnv_t[:], av_t[:], AF.Ln, bias=1.0)
                nc.scalar.activation(ar_t[:], ret_t[:], AF.Abs)
                nc.scalar.activation(lnr_t[:], ar_t[:], AF.Ln, bias=1.0)
                nc.scalar.activation(
                    ja_t[:], lp_t[:], AF.Copy, accum_out=acc_act[:, n : n + 1]
                )
                nc.scalar.activation(
                    ja_t[:], en_t[:], AF.Copy,
                    accum_out=acc_act[:, NT + n : NT + n + 1],
                )
                # DVE: signed symlog via sign-bit copy; Pool: difference
                _stt_uint_imm(
                    nc.vector, av_t[:].bitcast(u32), v_t[:].bitcast(u32),
                    SIGN_MASK, lnv_t[:].bitcast(u32),
                    OP.bitwise_and, OP.bitwise_or,
                )
                _stt_uint_imm(
                    nc.vector, ar_t[:].bitcast(u32), ret_t[:].bitcast(u32),
                    SIGN_MASK, lnr_t[:].bitcast(u32),
                    OP.bitwise_and, OP.bitwise_or,
                )
                nc.gpsimd.tensor_sub(d_t[:], av_t[:], ar_t[:])
                # ACT: sum((symlog v - symlog ret)^2)
                nc.scalar.activation(
                    ja_t[:], d_t[:], AF.Square,
                    accum_out=acc_act[:, 2 * NT + n : 2 * NT + n + 1],
                )

            nc.sync.dma_start(out_d[:, 0:N_DVE], acc_dve[:])
            nc.sync.dma_start(out_d[:, N_DVE:N_OUT], acc_act[:])

    nc.compile()
    return nc


_NC = None


def _get_nc():
    global _NC
    if _NC is None:
        _NC = build_module()
    return _NC


def _run(in_maps, trace=False, **kwargs):
    return run_bass_kernel_spmd(
        _get_nc(), in_maps, core_ids=list(range(NCORES)), trace=trace, **kwargs
    )


def make_in_maps(rewards, values, continues, bootstrap, log_probs, entropy):
    in_maps = []
    for i in range(NCORES):
        sl = slice(i * B_LOC, (i + 1) * B_LOC)
        in_maps.append(
            {
                "rewards": np.ascontiguousarray(rewards[sl], dtype=np.float32),
                "values": np.ascontiguousarray(values[sl], dtype=np.float32),
                "continues": np.ascontiguousarray(continues[sl], dtype=np.float32),
                "bootstrap": np.ascontiguousarray(bootstrap[sl], dtype=np.float32),
                "log_probs": np.ascontiguousarray(log_probs[sl], dtype=np.float32),
                "entropy": np.ascontiguousarray(entropy[sl], dtype=np.float32),
            }
        )
    return in_maps


def combine(results):
    """Host-side O(1) finish: global sums/extrema + EMA normalization."""
    outs = np.stack([res["out"] for res in results]).astype(np.float64)
    u1 = outs[:, :, 0:NT].sum()
    u2 = outs[:, :, NT : 2 * NT].sum()
    mx = outs[:, :, 2 * NT : 3 * NT].max()
    mn = outs[:, :, 3 * NT : 4 * NT].min()
    slp = outs[:, :, N_DVE : N_DVE + NT].sum()
    sent = outs[:, :, N_DVE + NT : N_DVE + 2 * NT].sum()
    d2 = outs[:, :, N_DVE + 2 * NT : N_DVE + 3 * NT].sum()

    n = float(B * T)
    ema = 1.0 - RETURN_EMA_DECAY
    lo_n = ema * mn                      # RETURN_LO0 = 0
    hi_n = 1.0 + ema * (mx - 1.0)        # RETURN_HI0 = 1
    scale = max(hi_n - lo_n, 1.0)
    # pg = -mean(lp * ((ret - lo_n)/scale - v))
    pg = -((u1 / n) / scale - lo_n * (slp / n) / scale - (u2 / n))
    entropy_loss = -ENTROPY_SCALE * (sent / n)
    critic = d2 / n
    return np.float32(pg + entropy_loss + critic)


def kernel(rewards, values, continues, bootstrap, log_probs, entropy):
    in_maps = make_in_maps(
        rewards, values, continues, bootstrap, log_probs, entropy
    )
    results = _run(in_maps).results
    return combine(results)
